# revision 1
# baseline (speedup 1.0000x reference)
"""Trainium2 Bass kernel for nn_ChunkwiseRecurrentAttentionCell.

Math (per (b,h) slice; T=256, Dk=Dv=128):
    gc = cumsum(g);  A = tril(beta_i exp(gc_i-gc_j) k_i.k_j, -1)
    v_new = (I+A)^{-1} (beta v - beta exp(gc) (k @ S0))
    out   = exp(gc) (q@S0) + (tril(exp(gc_i-gc_j),0) * (q k^T)) @ v_new
    S_new = exp(gc_T) S0 + k^T (v_new * exp(gc_T - gc))

Implemented as a chunked recurrence (2 chunks of 128) so all per-chunk exp
ratios are bounded by e^6.4 (fp16-safe).  The triangular solve uses an
8-term Neumann product form  (I+X^4)(I+X^2)(I+X), X = -A_chunk, with dual
power chains (both X^p and its transpose built by matmuls from masked
scalings of the symmetric K K^T — no big transposes needed).  All matmul
operands are fp16 (PE runs fp16 at 1 cycle/row vs fp32's 4); accumulation
is fp32 in PSUM.  Relative error vs the fp32 reference ~ 4e-4.

Sharding: (B,H) flattened to 512 independent slices, 64 per core across
8 NeuronCores (data parallel, no collectives).
"""

import os
import numpy as np

import concourse.bass as bass
import concourse.mybir as mybir
from concourse import bacc
from concourse.tile import TileContext
from concourse.masks import (
    make_identity,
    make_lower_triangular,
    make_upper_triangular,
)

B, H, T, DK, DV = 16, 32, 256, 128, 128
N_CORES = 8
N_SLICES = (B * H) // N_CORES  # 64 per core
CH = 128  # chunk length
N_CHUNKS = T // CH
LEVELS = 3  # Neumann product-form levels -> 2^3 = 8 series terms

F32 = mybir.dt.float32
MM_DT = mybir.dt.float16

_ALU = mybir.AluOpType
_ACTF = mybir.ActivationFunctionType


def build_nc(n_slices: int = N_SLICES):
    nc = bacc.Bacc("TRN2", target_bir_lowering=False)

    dq = nc.dram_tensor("q", [n_slices, T, DK], F32, kind="ExternalInput")
    dk = nc.dram_tensor("k", [n_slices, T, DK], F32, kind="ExternalInput")
    dv = nc.dram_tensor("v", [n_slices, T, DV], F32, kind="ExternalInput")
    dg = nc.dram_tensor("g", [n_slices, T], F32, kind="ExternalInput")
    db = nc.dram_tensor("beta", [n_slices, T], F32, kind="ExternalInput")
    ds0 = nc.dram_tensor("s0", [n_slices, DK, DV], F32, kind="ExternalInput")
    dout = nc.dram_tensor("out", [n_slices, T, DV], F32, kind="ExternalOutput")
    dsn = nc.dram_tensor("s_new", [n_slices, DK, DV], F32, kind="ExternalOutput")

    with TileContext(nc) as tc:
        with (
            tc.tile_pool(name="const", bufs=1) as cpool,
            tc.tile_pool(name="io", bufs=3) as iop,
            tc.tile_pool(name="ops", bufs=3) as opp,
            tc.tile_pool(name="state", bufs=2) as stp,
            tc.tile_pool(name="ps", bufs=1, space="PSUM") as psp,
        ):
            # ---------------- constants ----------------
            ident16 = cpool.tile([128, 128], MM_DT)
            make_identity(nc, ident16)
            ident32 = cpool.tile([128, 128], F32)
            make_identity(nc, ident32)
            mask_sl = cpool.tile([128, 128], F32)  # strict lower ones
            make_lower_triangular(nc, mask_sl, val=1.0, diag=False)
            mask_su = cpool.tile([128, 128], F32)  # strict upper ones
            make_upper_triangular(nc, mask_su, val=1.0, diag=False)
            mask_ui = cpool.tile([128, 128], F32)  # upper ones incl diag
            make_upper_triangular(nc, mask_ui, val=1.0, diag=True)

            # ---------------- per-core setup: gate vectors ----------------
            gt = cpool.tile([n_slices, T], F32)
            nc.sync.dma_start(gt[:], dg[:])
            bt = cpool.tile([n_slices, T], F32)
            nc.sync.dma_start(bt[:], db[:])
            gct = cpool.tile([n_slices, T], F32)
            nc.vector.tensor_tensor_scan(
                gct[:], gt[:], gt[:], 0.0, op0=_ALU.add, op1=_ALU.bypass
            )
            gcl1 = cpool.tile([n_slices, CH], F32)
            nc.vector.tensor_scalar(
                gcl1[:], gct[:, CH : 2 * CH], gct[:, CH - 1 : CH], None,
                op0=_ALU.subtract,
            )

            # per chunk: r, 1/r, -beta*r  in [n_slices, CH]; then transpose to
            # [CH, n_slices] so columns are per-slice partition-scalars.
            rT, irT, nbrT, bT, ET = [], [], [], [], []
            for c in range(N_CHUNKS):
                gcl = gct[:, 0:CH] if c == 0 else gcl1[:]
                r_c = cpool.tile([n_slices, CH], F32, name=f"r_{c}")
                nc.scalar.activation(r_c[:], gcl, _ACTF.Exp)
                ir_c = cpool.tile([n_slices, CH], F32, name=f"ir_{c}")
                nc.scalar.activation(ir_c[:], gcl, _ACTF.Exp, scale=-1.0)
                nbr_c = cpool.tile([n_slices, CH], F32, name=f"nbr_{c}")
                nc.vector.scalar_tensor_tensor(
                    nbr_c[:],
                    bt[:, c * CH : (c + 1) * CH],
                    -1.0,
                    r_c[:],
                    op0=_ALU.mult,
                    op1=_ALU.mult,
                )
                outs = []
                for src, nm in (
                    (r_c[:], "rT"),
                    (ir_c[:], "irT"),
                    (nbr_c[:], "nbrT"),
                    (bt[:, c * CH : (c + 1) * CH], "bT"),
                ):
                    pst = psp.tile([CH, n_slices], F32, name=f"pst_{nm}{c}", tag="ps_t", bufs=3)
                    nc.tensor.transpose(pst[:], src, ident32[0:n_slices, 0:n_slices])
                    dst = cpool.tile([CH, n_slices], F32, name=f"{nm}_{c}")
                    nc.scalar.copy(dst[:], pst[:])
                    outs.append(dst)
                rT.append(outs[0])
                irT.append(outs[1])
                nbrT.append(outs[2])
                bT.append(outs[3])
                ps_e = psp.tile([1, n_slices], F32, name=f"ps_e{c}", tag="ps_t", bufs=3)
                nc.tensor.transpose(
                    ps_e[:], r_c[:, CH - 1 : CH], ident32[0:n_slices, 0:n_slices]
                )
                e_row = cpool.tile([1, n_slices], F32, name=f"e_row_{c}")
                nc.scalar.copy(e_row[:], ps_e[:])
                e_c = cpool.tile([CH, n_slices], F32, name=f"ET_{c}")
                nc.gpsimd.partition_broadcast(e_c[:], e_row[0:1, :])
                ET.append(e_c)

            # ---------------- main loop over slices ----------------
            for s in range(n_slices):
                s_cur = None
                for c in range(N_CHUNKS):
                    tsl = slice(c * CH, (c + 1) * CH)
                    q_c = iop.tile([CH, DK], F32, name="q_c")
                    nc.sync.dma_start(q_c[:], dq[s, tsl, :])
                    k_c = iop.tile([CH, DK], F32, name="k_c")
                    nc.sync.dma_start(k_c[:], dk[s, tsl, :])
                    v_c = iop.tile([CH, DV], F32, name="v_c")
                    nc.sync.dma_start(v_c[:], dv[s, tsl, :])
                    if c == 0:
                        s_f32 = iop.tile([DK, DV], F32, name="s_f32")
                        nc.sync.dma_start(s_f32[:], ds0[s, :, :])
                        s_cur = stp.tile([DK, DV], MM_DT, name="s_cur")
                        nc.gpsimd.tensor_copy(s_cur[:], s_f32[:])

                    # scaled copies (fp16)
                    qr = opp.tile([CH, DK], MM_DT, name="qr")
                    nc.scalar.activation(
                        qr[:], q_c[:], _ACTF.Copy, scale=rT[c][:, s : s + 1]
                    )
                    knbr = opp.tile([CH, DK], MM_DT, name="knbr")
                    nc.vector.tensor_scalar_mul(knbr[:], k_c[:], nbrT[c][:, s : s + 1])
                    kir = opp.tile([CH, DK], MM_DT, name="kir")
                    nc.vector.tensor_scalar_mul(kir[:], k_c[:], irT[c][:, s : s + 1])

                    # transposes (PE) + copies (ACT)
                    qT = opp.tile([DK, CH], MM_DT, name="qT")
                    kTn = opp.tile([DK, CH], MM_DT, name="kTn")
                    kTi = opp.tile([DK, CH], MM_DT, name="kTi")
                    for src, dst, nm in ((qr, qT, "q"), (knbr, kTn, "n"), (kir, kTi, "i")):
                        ps_t = psp.tile([DK, CH], MM_DT, name=f"ps_t{nm}", tag="ps_t", bufs=3)
                        nc.tensor.transpose(ps_t[:], src[:], ident16[:])
                        nc.scalar.copy(dst[:], ps_t[:])

                    # Y = beta*v + (knbr @ S)     [= beta*v - beta*r*(k@S)]
                    ps_y = psp.tile([CH, DV], F32, name="ps_y", tag="mm", bufs=3)
                    nc.tensor.matmul(ps_y[:], kTn[:], s_cur[:])
                    z = opp.tile([CH, DV], MM_DT, name="z_it", tag="z", bufs=4)
                    nc.vector.scalar_tensor_tensor(
                        z[:], v_c[:], bT[c][:, s : s + 1], ps_y[:],
                        op0=_ALU.mult, op1=_ALU.add,
                    )

                    # B0 = -A = strict_tril(knbr @ kir^T); C0 = B0^T
                    ps_a = psp.tile([CH, CH], F32, name="ps_a", tag="mm", bufs=3)
                    nc.tensor.matmul(ps_a[:], kTn[:], kTi[:])
                    b0 = opp.tile([CH, CH], MM_DT, name="b0")
                    nc.vector.tensor_tensor(b0[:], ps_a[:], mask_sl[:], _ALU.mult)
                    ps_at = psp.tile([CH, CH], F32, name="ps_at", tag="mm", bufs=3)
                    nc.tensor.matmul(ps_at[:], kTi[:], kTn[:])
                    c0 = opp.tile([CH, CH], MM_DT, name="c0")
                    nc.vector.tensor_tensor(c0[:], ps_at[:], mask_su[:], _ALU.mult)

                    # dual chain: B1 = B0@B0, C1 = C0@C0, C2 = C1@C1
                    ps_b1 = psp.tile([CH, CH], F32, name="ps_b1", tag="mm", bufs=3)
                    nc.tensor.matmul(ps_b1[:], c0[:], b0[:])
                    b1 = opp.tile([CH, CH], MM_DT, name="b1")
                    nc.scalar.copy(b1[:], ps_b1[:])
                    ps_c1 = psp.tile([CH, CH], F32, name="ps_c1", tag="mm", bufs=3)
                    nc.tensor.matmul(ps_c1[:], b0[:], c0[:])
                    c1 = opp.tile([CH, CH], MM_DT, name="c1")
                    nc.scalar.copy(c1[:], ps_c1[:])
                    ps_c2 = psp.tile([CH, CH], F32, name="ps_c2", tag="mm", bufs=3)
                    nc.tensor.matmul(ps_c2[:], b1[:], c1[:])
                    c2 = opp.tile([CH, CH], MM_DT, name="c2")
                    nc.vector.tensor_copy(c2[:], ps_c2[:])

                    # applies: z <- z + X^(2^j) z   (lhsT = C_j)
                    for cj in (c0, c1, c2):
                        ps_ap = psp.tile([CH, DV], F32, name="ps_ap", tag="mm", bufs=3)
                        nc.tensor.matmul(ps_ap[:], cj[:], z[:])
                        z_new = opp.tile([CH, DV], MM_DT, name="z_new", tag="z", bufs=4)
                        nc.vector.tensor_tensor(z_new[:], ps_ap[:], z[:], _ALU.add)
                        z = z_new

                    # CQT = triu(kir @ qr^T, 0)
                    ps_cq = psp.tile([CH, CH], F32, name="ps_cq", tag="mm", bufs=3)
                    nc.tensor.matmul(ps_cq[:], kTi[:], qT[:])
                    cqt = opp.tile([CH, CH], MM_DT, name="cqt")
                    nc.vector.tensor_tensor(cqt[:], ps_cq[:], mask_ui[:], _ALU.mult)

                    # out = qr @ S + CQT^T @ z
                    ps_o = psp.tile([CH, DV], F32, name="ps_o", tag="ps_o", bufs=1)
                    nc.tensor.matmul(ps_o[:], qT[:], s_cur[:], start=True, stop=False)
                    nc.tensor.matmul(ps_o[:], cqt[:], z[:], start=False, stop=True)
                    o_sb = opp.tile([CH, DV], F32, name="o_sb")
                    nc.scalar.copy(o_sb[:], ps_o[:])
                    nc.sync.dma_start(dout[s, tsl, :], o_sb[:])

                    # state update: S' = E*(S + kir^T @ z)  [folded: Zs = E*z]
                    zs = opp.tile([CH, DV], MM_DT, name="zs")
                    nc.scalar.activation(
                        zs[:], z[:], _ACTF.Copy, scale=ET[c][:, s : s + 1]
                    )
                    ps_s = psp.tile([DK, DV], F32, name="ps_s", tag="ps_s", bufs=1)
                    nc.tensor.matmul(ps_s[:], kir[:], zs[:])
                    if c < N_CHUNKS - 1:
                        s_next = stp.tile([DK, DV], MM_DT, name="s_next")
                        nc.vector.scalar_tensor_tensor(
                            s_next[:], s_cur[:], ET[c][:, s : s + 1], ps_s[:],
                            op0=_ALU.mult, op1=_ALU.add,
                        )
                        s_cur = s_next
                    else:
                        s_fin = stp.tile([DK, DV], F32, name="s_fin")
                        nc.vector.scalar_tensor_tensor(
                            s_fin[:], s_cur[:], ET[c][:, s : s + 1], ps_s[:],
                            op0=_ALU.mult, op1=_ALU.add,
                        )
                        nc.sync.dma_start(dsn[s, :, :], s_fin[:])

    nc.compile()
    return nc


_NC_CACHE = {}


def _get_nc(n_slices):
    if n_slices not in _NC_CACHE:
        _NC_CACHE[n_slices] = build_nc(n_slices)
    return _NC_CACHE[n_slices]


def kernel(q, k, v, g, beta, last_recurrent_state):
    from concourse.bass_utils import run_bass_kernel_spmd

    qf = np.ascontiguousarray(q, np.float32).reshape(B * H, T, DK)
    kf = np.ascontiguousarray(k, np.float32).reshape(B * H, T, DK)
    vf = np.ascontiguousarray(v, np.float32).reshape(B * H, T, DV)
    gf = np.ascontiguousarray(g, np.float32).reshape(B * H, T)
    bf = np.ascontiguousarray(beta, np.float32).reshape(B * H, T)
    sf = np.ascontiguousarray(last_recurrent_state, np.float32).reshape(B * H, DK, DV)

    nc = _get_nc(N_SLICES)
    in_maps = []
    for i in range(N_CORES):
        sl = slice(i * N_SLICES, (i + 1) * N_SLICES)
        in_maps.append(
            {
                "q": qf[sl],
                "k": kf[sl],
                "v": vf[sl],
                "g": gf[sl],
                "beta": bf[sl],
                "s0": sf[sl],
            }
        )
    res = run_bass_kernel_spmd(nc, in_maps, list(range(N_CORES)))
    out = np.concatenate([res.results[i]["out"] for i in range(N_CORES)], axis=0)
    s_new = np.concatenate([res.results[i]["s_new"] for i in range(N_CORES)], axis=0)
    return np.concatenate([out.reshape(-1), s_new.reshape(-1)], axis=0)



# revision 6
# speedup vs baseline: 1.1909x; 1.1909x over previous
"""Trainium2 Bass kernel for nn_ChunkwiseRecurrentAttentionCell.

Math (per (b,h) slice; T=256, Dk=Dv=128), chunked by CH=128:
    gc = cumsum(g);  A = tril(beta_i exp(gc_i-gc_j) k_i.k_j, -1)
    v_new = (I+A)^{-1} (beta v - beta exp(gc) (k @ S))
    out   = exp(gc) (q@S) + (tril(exp(gc_i-gc_j),0) * (q k^T)) @ v_new
    S'    = e S + k^T (v_new * e exp(-gc)),   e = exp(sum g over chunk)

Key trick: similarity transform by D = diag(r_i * sqrt(beta_i)), r = exp(gc):
    A = D X D^{-1},  X = strict_tril(C'),  C' = kb kb^T  (SYMMETRIC),
    kb_i = sqrt(beta_i) k_i.
All exp-ratio column scalings vanish; the Neumann power chain needs only ONE
Gram product per chunk (C' symmetric -> X and X^T from the same PSUM tile via
two triangular masks).  With w = D^{-1} z:
    w     = v * (sqrt(beta)/r) - kb @ S           [one fused stt]
    z''   = (I+X)^{-1} w  via  (I+X)(I+X^2)(I+X^4) product form (8 terms)
    out   = r_i * [ q@S + triu(kb q^T,0)^T @ z'' ]  [r fold into PSUM copy]
    S'    = e S + kb^T (e z'')

Transposes (k_b^T, q^T) are plain matmuls against an fp16 identity (regular
matmul speed ~81ns, vs ~275ns transpose-mode).  Slices are processed in
groups of 4 ("quads") so every PSUM-consuming elementwise op runs on
[128, 512] tiles (one full PSUM bank), amortizing the fixed per-op overhead
4x.  Inputs stream in as fp16 via SWDGE cast-DMAs (gpsimd), outputs go out
as two batched f32 DMAs per quad on SP.

Sharding: (B,H) flattened to 512 slices, 64 per core on 8 NeuronCores
(data parallel, no collectives).
"""

import numpy as np

import concourse.bass as bass
import concourse.mybir as mybir
from concourse import bacc
from concourse.tile import TileContext
from concourse.masks import (
    make_identity,
    make_lower_triangular,
    make_upper_triangular,
)

B, H, T, DK, DV = 16, 32, 256, 128, 128
N_CORES = 8
N_SLICES = (B * H) // N_CORES  # 64 per core
CH = 128
N_CHUNKS = T // CH  # 2
QUAD = 4

F32 = mybir.dt.float32
F16 = mybir.dt.float16

_ALU = mybir.AluOpType
_ACTF = mybir.ActivationFunctionType


def build_nc(n_slices: int = N_SLICES, quad: int = QUAD):
    assert n_slices % quad == 0
    n_quads = n_slices // quad

    nc = bacc.Bacc("TRN2", target_bir_lowering=False)

    dq = nc.dram_tensor("q", [n_slices, T, DK], F32, kind="ExternalInput")
    dk = nc.dram_tensor("k", [n_slices, T, DK], F32, kind="ExternalInput")
    dv = nc.dram_tensor("v", [n_slices, T, DV], F32, kind="ExternalInput")
    dg = nc.dram_tensor("g", [n_slices, T], F32, kind="ExternalInput")
    db = nc.dram_tensor("beta", [n_slices, T], F32, kind="ExternalInput")
    ds0 = nc.dram_tensor("s0", [n_slices, DK, DV], F32, kind="ExternalInput")
    dout = nc.dram_tensor("out", [n_slices, T, DV], F32, kind="ExternalOutput")
    dsn = nc.dram_tensor("s_new", [n_slices, DK, DV], F32, kind="ExternalOutput")

    QW = quad * CH  # 512: quad-wide free dim

    with TileContext(nc) as tc:
        with (
            tc.tile_pool(name="const", bufs=1) as cpool,
            tc.tile_pool(name="io", bufs=2) as iop,
            tc.tile_pool(name="work", bufs=2) as wp,
            tc.tile_pool(name="ps", bufs=1, space="PSUM") as psp,
        ):
            # ---------------- constants ----------------
            ident16 = cpool.tile([128, 128], F16)
            make_identity(nc, ident16)
            ident32 = cpool.tile([128, 128], F32)
            make_identity(nc, ident32)
            # quad-wide triangular masks (4 copies side by side)
            msl = cpool.tile([128, QW], F32)  # strict lower, -1
            make_lower_triangular(nc, msl[:, 0:CH], val=-1.0, diag=False)
            msu = cpool.tile([128, QW], F32)  # strict upper, -1
            make_upper_triangular(nc, msu[:, 0:CH], val=-1.0, diag=False)
            mui = cpool.tile([128, QW], F32)  # upper incl diag, +1
            make_upper_triangular(nc, mui[:, 0:CH], val=1.0, diag=True)
            for m in (msl, msu, mui):
                nc.vector.tensor_copy(m[:, CH : 2 * CH], m[:, 0:CH])
                nc.vector.tensor_copy(m[:, 2 * CH : 4 * CH], m[:, 0 : 2 * CH])

            # ---------------- setup: per-(chunk, slice) gate vectors -------
            gt = cpool.tile([n_slices, T], F32)
            nc.sync.dma_start(gt[:], dg[:])
            bt = cpool.tile([n_slices, T], F32)
            nc.sync.dma_start(bt[:], db[:])
            gct = cpool.tile([n_slices, T], F32)
            nc.vector.tensor_tensor_scan(
                gct[:], gt[:], gt[:], 0.0, op0=_ALU.add, op1=_ALU.bypass
            )
            gcl1 = cpool.tile([n_slices, CH], F32)
            nc.vector.tensor_scalar(
                gcl1[:], gct[:, CH : 2 * CH], gct[:, CH - 1 : CH], None,
                op0=_ALU.subtract,
            )

            # per chunk, per slice column vectors [CH, n_slices]:
            #   rT    = exp(gcl)                (out scale)
            #   bsT   = sqrt(beta)              (kb scale)
            #   birT  = sqrt(beta) * exp(-gcl)  (w scale)
            #   ET    = exp(gcl[CH-1]) bcast    (state decay)
            rT, bsT, birT, ET = [], [], [], []
            for c in range(N_CHUNKS):
                gcl = gct[:, 0:CH] if c == 0 else gcl1[:]
                r_c = cpool.tile([n_slices, CH], F32, name=f"r_{c}")
                nc.scalar.activation(r_c[:], gcl, _ACTF.Exp)
                ir_c = cpool.tile([n_slices, CH], F32, name=f"ir_{c}")
                nc.scalar.activation(ir_c[:], gcl, _ACTF.Exp, scale=-1.0)
                bs_c = cpool.tile([n_slices, CH], F32, name=f"bs_{c}")
                nc.scalar.sqrt(bs_c[:], bt[:, c * CH : (c + 1) * CH])
                bir_c = cpool.tile([n_slices, CH], F32, name=f"bir_{c}")
                nc.vector.tensor_tensor(bir_c[:], bs_c[:], ir_c[:], _ALU.mult)

                outs = []
                for src, nm in ((r_c, "rT"), (bs_c, "bsT"), (bir_c, "birT")):
                    pst = psp.tile(
                        [CH, n_slices], F32, name=f"pst_{nm}{c}", tag="ps_t", bufs=3
                    )
                    nc.tensor.transpose(
                        pst[:], src[:], ident32[0:n_slices, 0:n_slices]
                    )
                    dst = cpool.tile([CH, n_slices], F32, name=f"{nm}_{c}")
                    nc.scalar.copy(dst[:], pst[:])
                    outs.append(dst)
                rT.append(outs[0])
                bsT.append(outs[1])
                birT.append(outs[2])

                ps_e = psp.tile([1, n_slices], F32, name=f"ps_e{c}", tag="ps_t", bufs=3)
                nc.tensor.transpose(
                    ps_e[:], r_c[:, CH - 1 : CH], ident32[0:n_slices, 0:n_slices]
                )
                e_row = cpool.tile([1, n_slices], F32, name=f"e_row_{c}")
                nc.scalar.copy(e_row[:], ps_e[:])
                e_c = cpool.tile([CH, n_slices], F32, name=f"ET_{c}")
                nc.gpsimd.partition_broadcast(e_c[:], e_row[0:1, :])
                ET.append(e_c)

            # ---------------- main loop over slice quads ----------------
            for qd in range(n_quads):
                qs = qd * quad

                # fp16 cast-loads via SWDGE: [128, quad*2*CH], quarters
                # laid out (slice, chunk): block (i*2+c) holds q[qs+i, chunk c]
                q16 = iop.tile([128, quad * T], F16, name="q16")
                nc.gpsimd.dma_start(
                    q16[:].rearrange("p (a c d) -> p a c d", a=quad, c=N_CHUNKS),
                    dq[qs : qs + quad].rearrange("a (c p) d -> p a c d", c=N_CHUNKS),
                )
                k16 = iop.tile([128, quad * T], F16, name="k16")
                nc.gpsimd.dma_start(
                    k16[:].rearrange("p (a c d) -> p a c d", a=quad, c=N_CHUNKS),
                    dk[qs : qs + quad].rearrange("a (c p) d -> p a c d", c=N_CHUNKS),
                )
                v16 = iop.tile([128, quad * T], F16, name="v16")
                nc.gpsimd.dma_start(
                    v16[:].rearrange("p (a c d) -> p a c d", a=quad, c=N_CHUNKS),
                    dv[qs : qs + quad].rearrange("a (c p) d -> p a c d", c=N_CHUNKS),
                )
                s0q = iop.tile([128, QW], F16, name="s0q")
                nc.gpsimd.dma_start(
                    s0q[:].rearrange("p (a d) -> p a d", a=quad),
                    ds0[qs : qs + quad].rearrange("a p d -> p a d"),
                )
                o_stage = iop.tile([128, quad * T], F32, name="o_stage")
                sn_stage = iop.tile([128, QW], F32, name="sn_stage")

                s_curs = [s0q[:, i * DV : (i + 1) * DV] for i in range(quad)]

                for c in range(N_CHUNKS):
                    def blk(tile, i, w=CH):
                        # block (slice i, chunk c) of an io quad tile
                        j = i * N_CHUNKS + c
                        return tile[:, j * w : (j + 1) * w]

                    def cvec(vecs, i):
                        return vecs[c][:, qs + i : qs + i + 1]

                    # kb = sqrt(beta) * k   (fp16, natural layout) [ACT]
                    kb16 = wp.tile([128, QW], F16, name="kb16", tag="kb16", bufs=3)
                    for i in range(quad):
                        nc.scalar.activation(
                            kb16[:, i * DK : (i + 1) * DK], blk(k16, i),
                            _ACTF.Copy, scale=cvec(bsT, i),
                        )

                    # transposes as plain matmuls vs identity -> f32 PSUM
                    # (pair tiles: [kbT_i | qT_i | kbT_i+1 | qT_i+1], one bank)
                    tsb = wp.tile([128, 2 * QW], F16, name="tsb", tag="tsb", bufs=3)
                    for p in range(quad // 2):
                        ps_t = psp.tile(
                            [128, 512], F32, name=f"ps_t{p}", tag="ps_t", bufs=3
                        )
                        for j in range(2):
                            i = 2 * p + j
                            nc.tensor.matmul(
                                ps_t[:, (2 * j) * CH : (2 * j + 1) * CH],
                                kb16[:, i * DK : (i + 1) * DK], ident16[:],
                            )
                            nc.tensor.matmul(
                                ps_t[:, (2 * j + 1) * CH : (2 * j + 2) * CH],
                                blk(q16, i), ident16[:],
                            )
                        nc.scalar.copy(
                            tsb[:, p * 512 : (p + 1) * 512], ps_t[:]
                        )
                    kbT = [tsb[:, (2 * i) * CH : (2 * i + 1) * CH] for i in range(quad)]
                    qT = [
                        tsb[:, (2 * i + 1) * CH : (2 * i + 2) * CH]
                        for i in range(quad)
                    ]

                    # C' = kb kb^T (symmetric Gram), quad-batched PSUM
                    ps_c = psp.tile([128, QW], F32, name="ps_c", tag="mmq", bufs=3)
                    for i in range(quad):
                        nc.tensor.matmul(
                            ps_c[:, i * CH : (i + 1) * CH], kbT[i], kbT[i]
                        )
                    b0 = wp.tile([128, QW], F16, name="b0", tag="b0", bufs=2)
                    nc.vector.tensor_tensor(b0[:], ps_c[:], msl[:], _ALU.mult)
                    c0 = wp.tile([128, QW], F16, name="c0", tag="c0", bufs=2)
                    nc.vector.tensor_tensor(c0[:], ps_c[:], msu[:], _ALU.mult)

                    # w = v*(sqrt(beta)/r) - kb@S
                    ps_y = psp.tile([128, QW], F32, name="ps_y", tag="mmq", bufs=3)
                    for i in range(quad):
                        nc.tensor.matmul(
                            ps_y[:, i * DV : (i + 1) * DV], kbT[i], s_curs[i]
                        )
                    z = wp.tile([128, QW], F16, name="z0", tag="z", bufs=8)
                    for i in range(quad):
                        nc.vector.scalar_tensor_tensor(
                            z[:, i * DV : (i + 1) * DV], blk(v16, i),
                            cvec(birT, i), ps_y[:, i * DV : (i + 1) * DV],
                            op0=_ALU.mult, op1=_ALU.subtract,
                        )

                    # power chain: b1 = X^2, c1 = (X^2)^T (pairs), c2 = (X^4)^T
                    b1c1 = []
                    for p in range(quad // 2):
                        psb = psp.tile(
                            [128, 512], F32, name=f"psb{p}", tag="b1c1", bufs=2
                        )
                        for j in range(2):
                            i = 2 * p + j
                            bsl = b0[:, i * CH : (i + 1) * CH]
                            csl = c0[:, i * CH : (i + 1) * CH]
                            nc.tensor.matmul(
                                psb[:, (2 * j) * CH : (2 * j + 1) * CH], csl, bsl
                            )
                            nc.tensor.matmul(
                                psb[:, (2 * j + 1) * CH : (2 * j + 2) * CH], bsl, csl
                            )
                        sb = wp.tile(
                            [128, 512], F16, name=f"b1c1_{p}", tag="b1c1sb", bufs=4
                        )
                        nc.scalar.copy(sb[:], psb[:])
                        b1c1.append(sb)

                    def b1sl(i):
                        return b1c1[i // 2][:, (2 * (i % 2)) * CH : (2 * (i % 2) + 1) * CH]

                    def c1sl(i):
                        return b1c1[i // 2][:, (2 * (i % 2) + 1) * CH : (2 * (i % 2) + 2) * CH]

                    ps_c2 = psp.tile([128, QW], F32, name="ps_c2", tag="mmq", bufs=3)
                    for i in range(quad):
                        nc.tensor.matmul(
                            ps_c2[:, i * CH : (i + 1) * CH], b1sl(i), c1sl(i)
                        )
                    c2 = wp.tile([128, QW], F16, name="c2", tag="c2", bufs=2)
                    nc.scalar.copy(c2[:], ps_c2[:])

                    # applies: z <- z + X^(2^j) z
                    for mats in (
                        [c0[:, i * CH : (i + 1) * CH] for i in range(quad)],
                        [c1sl(i) for i in range(quad)],
                        [c2[:, i * CH : (i + 1) * CH] for i in range(quad)],
                    ):
                        ps_ap = psp.tile(
                            [128, QW], F32, name="ps_ap", tag="mmq", bufs=3
                        )
                        for i in range(quad):
                            nc.tensor.matmul(
                                ps_ap[:, i * DV : (i + 1) * DV], mats[i],
                                z[:, i * DV : (i + 1) * DV],
                            )
                        z_new = wp.tile([128, QW], F16, name="z_n", tag="z", bufs=8)
                        nc.vector.tensor_tensor(z_new[:], ps_ap[:], z[:], _ALU.add)
                        z = z_new

                    # cqt = triu(kb q^T, 0)  (lhsT for the intra apply)
                    ps_cq = psp.tile([128, QW], F32, name="ps_cq", tag="mmq", bufs=3)
                    for i in range(quad):
                        nc.tensor.matmul(
                            ps_cq[:, i * CH : (i + 1) * CH], kbT[i], qT[i]
                        )
                    cqt = wp.tile([128, QW], F16, name="cqt", tag="cqt", bufs=2)
                    nc.vector.tensor_tensor(cqt[:], ps_cq[:], mui[:], _ALU.mult)

                    # out = r * (q@S + cqt^T @ z)
                    ps_o = psp.tile([128, QW], F32, name="ps_o", tag="mmq", bufs=3)
                    for i in range(quad):
                        nc.tensor.matmul(
                            ps_o[:, i * DV : (i + 1) * DV], qT[i], s_curs[i],
                            start=True, stop=False,
                        )
                        nc.tensor.matmul(
                            ps_o[:, i * DV : (i + 1) * DV],
                            cqt[:, i * CH : (i + 1) * CH],
                            z[:, i * DV : (i + 1) * DV],
                            start=False, stop=True,
                        )
                    for i in range(quad):
                        nc.scalar.activation(
                            blk(o_stage, i, DV), ps_o[:, i * DV : (i + 1) * DV],
                            _ACTF.Copy, scale=cvec(rT, i),
                        )

                    # state: S' = e*S + kb^T (e z)
                    zs = wp.tile([128, QW], F16, name="zs", tag="zs", bufs=2)
                    for i in range(quad):
                        nc.gpsimd.tensor_scalar_mul(
                            zs[:, i * DV : (i + 1) * DV],
                            z[:, i * DV : (i + 1) * DV], cvec(ET, i),
                        )
                    ps_s = psp.tile([128, QW], F32, name="ps_s", tag="mmq", bufs=3)
                    for i in range(quad):
                        nc.tensor.matmul(
                            ps_s[:, i * DV : (i + 1) * DV],
                            kb16[:, i * DK : (i + 1) * DK],
                            zs[:, i * DV : (i + 1) * DV],
                        )
                    if c < N_CHUNKS - 1:
                        s_next = wp.tile(
                            [128, QW], F16, name="s_next", tag="s_next", bufs=2
                        )
                        for i in range(quad):
                            nc.vector.scalar_tensor_tensor(
                                s_next[:, i * DV : (i + 1) * DV], s_curs[i],
                                cvec(ET, i), ps_s[:, i * DV : (i + 1) * DV],
                                op0=_ALU.mult, op1=_ALU.add,
                            )
                        s_curs = [
                            s_next[:, i * DV : (i + 1) * DV] for i in range(quad)
                        ]
                    else:
                        for i in range(quad):
                            nc.vector.scalar_tensor_tensor(
                                sn_stage[:, i * DV : (i + 1) * DV], s_curs[i],
                                cvec(ET, i), ps_s[:, i * DV : (i + 1) * DV],
                                op0=_ALU.mult, op1=_ALU.add,
                            )

                nc.sync.dma_start(
                    dout[qs : qs + quad].rearrange("a (c p) d -> p a c d", c=N_CHUNKS),
                    o_stage[:].rearrange("p (a c d) -> p a c d", a=quad, c=N_CHUNKS),
                )
                nc.sync.dma_start(
                    dsn[qs : qs + quad].rearrange("a p d -> p a d"),
                    sn_stage[:].rearrange("p (a d) -> p a d", a=quad),
                )

    nc.compile()
    return nc


_NC_CACHE = {}


def _get_nc(n_slices):
    if n_slices not in _NC_CACHE:
        _NC_CACHE[n_slices] = build_nc(n_slices)
    return _NC_CACHE[n_slices]


def kernel(q, k, v, g, beta, last_recurrent_state):
    from concourse.bass_utils import run_bass_kernel_spmd

    qf = np.ascontiguousarray(q, np.float32).reshape(B * H, T, DK)
    kf = np.ascontiguousarray(k, np.float32).reshape(B * H, T, DK)
    vf = np.ascontiguousarray(v, np.float32).reshape(B * H, T, DV)
    gf = np.ascontiguousarray(g, np.float32).reshape(B * H, T)
    bf = np.ascontiguousarray(beta, np.float32).reshape(B * H, T)
    sf = np.ascontiguousarray(last_recurrent_state, np.float32).reshape(B * H, DK, DV)

    nc = _get_nc(N_SLICES)
    in_maps = []
    for i in range(N_CORES):
        sl = slice(i * N_SLICES, (i + 1) * N_SLICES)
        in_maps.append(
            {
                "q": qf[sl],
                "k": kf[sl],
                "v": vf[sl],
                "g": gf[sl],
                "beta": bf[sl],
                "s0": sf[sl],
            }
        )
    res = run_bass_kernel_spmd(nc, in_maps, list(range(N_CORES)))
    out = np.concatenate([res.results[i]["out"] for i in range(N_CORES)], axis=0)
    s_new = np.concatenate([res.results[i]["s_new"] for i in range(N_CORES)], axis=0)
    return np.concatenate([out.reshape(-1), s_new.reshape(-1)], axis=0)


# revision 10
# speedup vs baseline: 1.9228x; 1.6146x over previous
"""Trainium2 Bass kernel for nn_ChunkwiseRecurrentAttentionCell.

Math (per (b,h) slice; T=256, Dk=Dv=128), chunked by CH=128:
    gc = cumsum(g);  A = tril(beta_i exp(gc_i-gc_j) k_i.k_j, -1)
    v_new = (I+A)^{-1} (beta v - beta exp(gc) (k @ S))
    out   = exp(gc) (q@S) + (tril(exp(gc_i-gc_j),0) * (q k^T)) @ v_new
    S'    = e S + k^T (v_new * e exp(-gc)),   e = exp(sum g over chunk)

Key trick: similarity transform by D = diag(r_i * sqrt(beta_i)), r = exp(gc):
    A = D X D^{-1},  X = strict_tril(C'),  C' = kb kb^T  (SYMMETRIC),
    kb_i = sqrt(beta_i) k_i.
All exp-ratio column scalings vanish; the Neumann power chain needs only ONE
Gram product per chunk (C' symmetric -> X and X^T from the same PSUM tile via
two triangular masks).  With w = D^{-1} z:
    w     = v * (sqrt(beta)/r) - kb @ S           [one fused stt]
    z''   = (I+X)^{-1} w  via  (I+X)(I+X^2)(I+X^4) product form (8 terms)
    out   = r_i * [ q@S + triu(kb q^T,0)^T @ z'' ]  [r fold into PSUM copy]
    S'    = e S + kb^T (e z'')

Transposes (k_b^T, q^T) are plain matmuls against an fp16 identity (regular
matmul speed ~81ns, vs ~275ns transpose-mode).  Slices are processed in
groups of 4 ("quads") so every PSUM-consuming elementwise op runs on
[128, 512] tiles (one full PSUM bank), amortizing the fixed per-op overhead
4x.  Inputs stream in as fp16 via SWDGE cast-DMAs (gpsimd), outputs go out
as two batched f32 DMAs per quad on SP.

Sharding: (B,H) flattened to 512 slices, 64 per core on 8 NeuronCores
(data parallel, no collectives).
"""

import numpy as np

import concourse.bass as bass
import concourse.mybir as mybir
from concourse import bacc
from concourse.tile import TileContext
from concourse.masks import (
    make_identity,
    make_lower_triangular,
    make_upper_triangular,
)

B, H, T, DK, DV = 16, 32, 256, 128, 128
N_CORES = 8
N_SLICES = (B * H) // N_CORES  # 64 per core
CH = 128
N_CHUNKS = T // CH  # 2
QUAD = 4

F32 = mybir.dt.float32
F16 = mybir.dt.float16

_ALU = mybir.AluOpType
_ACTF = mybir.ActivationFunctionType


def build_nc(n_slices: int = N_SLICES, quad: int = QUAD):
    assert n_slices % quad == 0
    n_quads = n_slices // quad

    nc = bacc.Bacc("TRN2", target_bir_lowering=False)

    dq = nc.dram_tensor("q", [n_slices, T, DK], F32, kind="ExternalInput")
    dk = nc.dram_tensor("k", [n_slices, T, DK], F32, kind="ExternalInput")
    dv = nc.dram_tensor("v", [n_slices, T, DV], F32, kind="ExternalInput")
    dg = nc.dram_tensor("g", [n_slices, T], F32, kind="ExternalInput")
    db = nc.dram_tensor("beta", [n_slices, T], F32, kind="ExternalInput")
    ds0 = nc.dram_tensor("s0", [n_slices, DK, DV], F32, kind="ExternalInput")
    dout = nc.dram_tensor("out", [n_slices, T, DV], F32, kind="ExternalOutput")
    dsn = nc.dram_tensor("s_new", [n_slices, DK, DV], F32, kind="ExternalOutput")

    QW = quad * CH  # 512: quad-wide free dim

    with TileContext(nc) as tc:
        with (
            tc.tile_pool(name="const", bufs=1) as cpool,
            tc.tile_pool(name="io", bufs=3) as iop,
            tc.tile_pool(name="work", bufs=2) as wp,
            tc.tile_pool(name="ps", bufs=1, space="PSUM") as psp,
        ):
            # ---------------- constants ----------------
            ident16 = cpool.tile([128, 128], F16)
            make_identity(nc, ident16)
            ident32 = cpool.tile([128, 128], F32)
            make_identity(nc, ident32)
            # quad-wide triangular masks (4 copies side by side)
            msl = cpool.tile([128, QW], F32)  # strict lower, -1
            make_lower_triangular(nc, msl[:, 0:CH], val=-1.0, diag=False)
            msu = cpool.tile([128, QW], F32)  # strict upper, -1
            make_upper_triangular(nc, msu[:, 0:CH], val=-1.0, diag=False)
            mui = cpool.tile([128, QW], F32)  # upper incl diag, +1
            make_upper_triangular(nc, mui[:, 0:CH], val=1.0, diag=True)
            for m in (msl, msu, mui):
                nc.vector.tensor_copy(m[:, CH : 2 * CH], m[:, 0:CH])
                nc.vector.tensor_copy(m[:, 2 * CH : 4 * CH], m[:, 0 : 2 * CH])

            # ---------------- setup: per-(chunk, slice) gate vectors -------
            gt = cpool.tile([n_slices, T], F32)
            nc.sync.dma_start(gt[:], dg[:])
            bt = cpool.tile([n_slices, T], F32)
            nc.sync.dma_start(bt[:], db[:])
            gct = cpool.tile([n_slices, T], F32)
            nc.vector.tensor_tensor_scan(
                gct[:], gt[:], gt[:], 0.0, op0=_ALU.add, op1=_ALU.bypass
            )
            gcl1 = cpool.tile([n_slices, CH], F32)
            nc.vector.tensor_scalar(
                gcl1[:], gct[:, CH : 2 * CH], gct[:, CH - 1 : CH], None,
                op0=_ALU.subtract,
            )

            # per chunk, per slice column vectors [CH, n_slices]:
            #   rT    = exp(gcl)                (out scale)
            #   bsT   = sqrt(beta)              (kb scale)
            #   birT  = sqrt(beta) * exp(-gcl)  (w scale)
            #   ET    = exp(gcl[CH-1]) bcast    (state decay)
            rT, bsT, birT, ET = [], [], [], []
            for c in range(N_CHUNKS):
                gcl = gct[:, 0:CH] if c == 0 else gcl1[:]
                r_c = cpool.tile([n_slices, CH], F32, name=f"r_{c}")
                nc.scalar.activation(r_c[:], gcl, _ACTF.Exp)
                ir_c = cpool.tile([n_slices, CH], F32, name=f"ir_{c}")
                nc.scalar.activation(ir_c[:], gcl, _ACTF.Exp, scale=-1.0)
                bs_c = cpool.tile([n_slices, CH], F32, name=f"bs_{c}")
                nc.scalar.sqrt(bs_c[:], bt[:, c * CH : (c + 1) * CH])
                bir_c = cpool.tile([n_slices, CH], F32, name=f"bir_{c}")
                nc.vector.tensor_tensor(bir_c[:], bs_c[:], ir_c[:], _ALU.mult)

                outs = []
                for src, nm in ((r_c, "rT"), (bs_c, "bsT"), (bir_c, "birT")):
                    pst = psp.tile(
                        [CH, n_slices], F32, name=f"pst_{nm}{c}", tag="ps_t", bufs=2
                    )
                    nc.tensor.transpose(
                        pst[:], src[:], ident32[0:n_slices, 0:n_slices]
                    )
                    dst = cpool.tile([CH, n_slices], F32, name=f"{nm}_{c}")
                    nc.scalar.copy(dst[:], pst[:])
                    outs.append(dst)
                rT.append(outs[0])
                bsT.append(outs[1])
                birT.append(outs[2])

                ps_e = psp.tile([1, n_slices], F32, name=f"ps_e{c}", tag="ps_t", bufs=2)
                nc.tensor.transpose(
                    ps_e[:], r_c[:, CH - 1 : CH], ident32[0:n_slices, 0:n_slices]
                )
                e_row = cpool.tile([1, n_slices], F32, name=f"e_row_{c}")
                nc.scalar.copy(e_row[:], ps_e[:])
                e_c = cpool.tile([CH, n_slices], F32, name=f"ET_{c}")
                nc.gpsimd.partition_broadcast(e_c[:], e_row[0:1, :])
                ET.append(e_c)

            # ---------------- main loop over slice quads ----------------
            for qd in range(n_quads):
                qs = qd * quad

                # fp16 cast-loads via SWDGE: [128, quad*2*CH], quarters
                # laid out (slice, chunk): block (i*2+c) holds q[qs+i, chunk c]
                q16 = iop.tile([128, quad * T], F16, name="q16")
                nc.gpsimd.dma_start(
                    q16[:].rearrange("p (a c d) -> p a c d", a=quad, c=N_CHUNKS),
                    dq[qs : qs + quad].rearrange("a (c p) d -> p a c d", c=N_CHUNKS),
                )
                k16 = iop.tile([128, quad * T], F16, name="k16")
                nc.gpsimd.dma_start(
                    k16[:].rearrange("p (a c d) -> p a c d", a=quad, c=N_CHUNKS),
                    dk[qs : qs + quad].rearrange("a (c p) d -> p a c d", c=N_CHUNKS),
                )
                v16 = iop.tile([128, quad * T], F16, name="v16")
                nc.gpsimd.dma_start(
                    v16[:].rearrange("p (a c d) -> p a c d", a=quad, c=N_CHUNKS),
                    dv[qs : qs + quad].rearrange("a (c p) d -> p a c d", c=N_CHUNKS),
                )
                s0q = iop.tile([128, QW], F16, name="s0q")
                nc.gpsimd.dma_start(
                    s0q[:].rearrange("p (a d) -> p a d", a=quad),
                    ds0[qs : qs + quad].rearrange("a p d -> p a d"),
                )
                o_stage = iop.tile([128, quad * T], F32, name="o_stage")
                sn_stage = iop.tile([128, QW], F32, name="sn_stage")

                s_curs = [s0q[:, i * DV : (i + 1) * DV] for i in range(quad)]

                for c in range(N_CHUNKS):
                    def blk(tile, i, w=CH):
                        # block (slice i, chunk c) of an io quad tile
                        j = i * N_CHUNKS + c
                        return tile[:, j * w : (j + 1) * w]

                    def cvec(vecs, i):
                        return vecs[c][:, qs + i : qs + i + 1]

                    # kb = sqrt(beta) * k   (fp16, natural layout) [ACT]
                    kb16 = wp.tile([128, QW], F16, name="kb16", tag="kb16", bufs=3)
                    for i in range(quad):
                        nc.scalar.activation(
                            kb16[:, i * DK : (i + 1) * DK], blk(k16, i),
                            _ACTF.Copy, scale=cvec(bsT, i),
                        )
                    # qr = r * q  (folds the output row-scale into the q side,
                    # so the out copy is one quad-wide plain copy) [DVE]
                    qr16 = wp.tile([128, QW], F16, name="qr16", tag="qr16", bufs=3)
                    for i in range(quad):
                        nc.vector.tensor_scalar_mul(
                            qr16[:, i * DK : (i + 1) * DK], blk(q16, i), cvec(rT, i)
                        )

                    # transposes as plain matmuls vs identity -> f32 PSUM
                    # (pair tiles: [kbT_i | qT_i | kbT_i+1 | qT_i+1], one bank)
                    tsb = wp.tile([128, 2 * QW], F16, name="tsb", tag="tsb", bufs=3)
                    for p in range(quad // 2):
                        ps_t = psp.tile(
                            [128, 512], F32, name=f"ps_t{p}", tag="ps_t", bufs=2
                        )
                        for j in range(2):
                            i = 2 * p + j
                            nc.tensor.matmul(
                                ps_t[:, (2 * j) * CH : (2 * j + 1) * CH],
                                kb16[:, i * DK : (i + 1) * DK], ident16[:],
                            )
                            nc.tensor.matmul(
                                ps_t[:, (2 * j + 1) * CH : (2 * j + 2) * CH],
                                qr16[:, i * DK : (i + 1) * DK], ident16[:],
                            )
                        nc.scalar.copy(
                            tsb[:, p * 512 : (p + 1) * 512], ps_t[:]
                        )
                    kbT = [tsb[:, (2 * i) * CH : (2 * i + 1) * CH] for i in range(quad)]
                    qT = [
                        tsb[:, (2 * i + 1) * CH : (2 * i + 2) * CH]
                        for i in range(quad)
                    ]

                    # C' = kb kb^T (symmetric Gram), quad-batched PSUM
                    ps_c = psp.tile([128, QW], F32, name="ps_c", tag="mmq", bufs=6)
                    for i in range(quad):
                        nc.tensor.matmul(
                            ps_c[:, i * CH : (i + 1) * CH], kbT[i], kbT[i]
                        )
                    b0 = wp.tile([128, QW], F16, name="b0", tag="b0", bufs=3)
                    nc.vector.tensor_tensor(b0[:], ps_c[:], msl[:], _ALU.mult)
                    c0 = wp.tile([128, QW], F16, name="c0", tag="c0", bufs=3)
                    nc.vector.tensor_tensor(c0[:], ps_c[:], msu[:], _ALU.mult)

                    # w = v*(sqrt(beta)/r) - kb@S
                    ps_y = psp.tile([128, QW], F32, name="ps_y", tag="mmq", bufs=6)
                    for i in range(quad):
                        nc.tensor.matmul(
                            ps_y[:, i * DV : (i + 1) * DV], kbT[i], s_curs[i]
                        )
                    z = wp.tile([128, QW], F16, name="z0", tag="z", bufs=8)
                    for i in range(quad):
                        nc.vector.scalar_tensor_tensor(
                            z[:, i * DV : (i + 1) * DV], blk(v16, i),
                            cvec(birT, i), ps_y[:, i * DV : (i + 1) * DV],
                            op0=_ALU.mult, op1=_ALU.subtract,
                        )

                    # power chain: b1 = X^2, c1 = (X^2)^T (pairs), c2 = (X^4)^T
                    b1c1 = []
                    for p in range(quad // 2):
                        psb = psp.tile(
                            [128, 512], F32, name=f"psb{p}", tag="mmq", bufs=6
                        )
                        for j in range(2):
                            i = 2 * p + j
                            bsl = b0[:, i * CH : (i + 1) * CH]
                            csl = c0[:, i * CH : (i + 1) * CH]
                            nc.tensor.matmul(
                                psb[:, (2 * j) * CH : (2 * j + 1) * CH], csl, bsl
                            )
                            nc.tensor.matmul(
                                psb[:, (2 * j + 1) * CH : (2 * j + 2) * CH], bsl, csl
                            )
                        sb = wp.tile(
                            [128, 512], F16, name=f"b1c1_{p}", tag="b1c1sb", bufs=6
                        )
                        nc.scalar.copy(sb[:], psb[:])
                        b1c1.append(sb)

                    def b1sl(i):
                        return b1c1[i // 2][:, (2 * (i % 2)) * CH : (2 * (i % 2) + 1) * CH]

                    def c1sl(i):
                        return b1c1[i // 2][:, (2 * (i % 2) + 1) * CH : (2 * (i % 2) + 2) * CH]

                    ps_c2 = psp.tile([128, QW], F32, name="ps_c2", tag="mmq", bufs=6)
                    for i in range(quad):
                        nc.tensor.matmul(
                            ps_c2[:, i * CH : (i + 1) * CH], b1sl(i), c1sl(i)
                        )
                    c2 = wp.tile([128, QW], F16, name="c2", tag="c2", bufs=3)
                    nc.scalar.copy(c2[:], ps_c2[:])

                    # applies: z <- z + X^(2^j) z
                    for mats in (
                        [c0[:, i * CH : (i + 1) * CH] for i in range(quad)],
                        [c1sl(i) for i in range(quad)],
                        [c2[:, i * CH : (i + 1) * CH] for i in range(quad)],
                    ):
                        ps_ap = psp.tile(
                            [128, QW], F32, name="ps_ap", tag="mmq", bufs=6
                        )
                        for i in range(quad):
                            nc.tensor.matmul(
                                ps_ap[:, i * DV : (i + 1) * DV], mats[i],
                                z[:, i * DV : (i + 1) * DV],
                            )
                        z_new = wp.tile([128, QW], F16, name="z_n", tag="z", bufs=8)
                        nc.vector.tensor_tensor(z_new[:], ps_ap[:], z[:], _ALU.add)
                        z = z_new

                    # state first (critical path to the next chunk):
                    # S' = e*S + kb^T (e z)
                    zs = wp.tile([128, QW], F16, name="zs", tag="zs", bufs=3)
                    for i in range(quad):
                        nc.vector.tensor_scalar_mul(
                            zs[:, i * DV : (i + 1) * DV],
                            z[:, i * DV : (i + 1) * DV], cvec(ET, i),
                        )
                    ps_s = psp.tile([128, QW], F32, name="ps_s", tag="mmq", bufs=6)
                    for i in range(quad):
                        nc.tensor.matmul(
                            ps_s[:, i * DV : (i + 1) * DV],
                            kb16[:, i * DK : (i + 1) * DK],
                            zs[:, i * DV : (i + 1) * DV],
                        )
                    if c < N_CHUNKS - 1:
                        s_next = wp.tile(
                            [128, QW], F16, name="s_next", tag="s_next", bufs=2
                        )
                        for i in range(quad):
                            nc.vector.scalar_tensor_tensor(
                                s_next[:, i * DV : (i + 1) * DV], s_curs[i],
                                cvec(ET, i), ps_s[:, i * DV : (i + 1) * DV],
                                op0=_ALU.mult, op1=_ALU.add,
                            )
                        new_s_curs = [
                            s_next[:, i * DV : (i + 1) * DV] for i in range(quad)
                        ]
                    else:
                        for i in range(quad):
                            nc.vector.scalar_tensor_tensor(
                                sn_stage[:, i * DV : (i + 1) * DV], s_curs[i],
                                cvec(ET, i), ps_s[:, i * DV : (i + 1) * DV],
                                op0=_ALU.mult, op1=_ALU.add,
                            )
                        new_s_curs = None

                    # cqt = triu(kb (r q)^T, 0)  (lhsT for the intra apply)
                    ps_cq = psp.tile([128, QW], F32, name="ps_cq", tag="mmq", bufs=6)
                    for i in range(quad):
                        nc.tensor.matmul(
                            ps_cq[:, i * CH : (i + 1) * CH], kbT[i], qT[i]
                        )
                    cqt = wp.tile([128, QW], F16, name="cqt", tag="cqt", bufs=3)
                    nc.vector.tensor_tensor(cqt[:], ps_cq[:], mui[:], _ALU.mult)

                    # out = r q@S + cqt^T @ z  (r already folded into qT)
                    ps_o = psp.tile([128, QW], F32, name="ps_o", tag="mmq", bufs=6)
                    for i in range(quad):
                        nc.tensor.matmul(
                            ps_o[:, i * DV : (i + 1) * DV], qT[i], s_curs[i],
                            start=True, stop=False,
                        )
                        nc.tensor.matmul(
                            ps_o[:, i * DV : (i + 1) * DV],
                            cqt[:, i * CH : (i + 1) * CH],
                            z[:, i * DV : (i + 1) * DV],
                            start=False, stop=True,
                        )
                    nc.scalar.copy(
                        o_stage[:].rearrange(
                            "p (a c d) -> p a c d", a=quad, c=N_CHUNKS
                        )[:, :, c, :],
                        ps_o[:].rearrange("p (a d) -> p a d", a=quad),
                    )

                    if new_s_curs is not None:
                        s_curs = new_s_curs

                nc.sync.dma_start(
                    dout[qs : qs + quad].rearrange("a (c p) d -> p a c d", c=N_CHUNKS),
                    o_stage[:].rearrange("p (a c d) -> p a c d", a=quad, c=N_CHUNKS),
                )
                nc.sync.dma_start(
                    dsn[qs : qs + quad].rearrange("a p d -> p a d"),
                    sn_stage[:].rearrange("p (a d) -> p a d", a=quad),
                )

    nc.compile()
    return nc


_NC_CACHE = {}


def _get_nc(n_slices):
    if n_slices not in _NC_CACHE:
        _NC_CACHE[n_slices] = build_nc(n_slices)
    return _NC_CACHE[n_slices]


def kernel(q, k, v, g, beta, last_recurrent_state):
    from concourse.bass_utils import run_bass_kernel_spmd

    qf = np.ascontiguousarray(q, np.float32).reshape(B * H, T, DK)
    kf = np.ascontiguousarray(k, np.float32).reshape(B * H, T, DK)
    vf = np.ascontiguousarray(v, np.float32).reshape(B * H, T, DV)
    gf = np.ascontiguousarray(g, np.float32).reshape(B * H, T)
    bf = np.ascontiguousarray(beta, np.float32).reshape(B * H, T)
    sf = np.ascontiguousarray(last_recurrent_state, np.float32).reshape(B * H, DK, DV)

    nc = _get_nc(N_SLICES)
    in_maps = []
    for i in range(N_CORES):
        sl = slice(i * N_SLICES, (i + 1) * N_SLICES)
        in_maps.append(
            {
                "q": qf[sl],
                "k": kf[sl],
                "v": vf[sl],
                "g": gf[sl],
                "beta": bf[sl],
                "s0": sf[sl],
            }
        )
    res = run_bass_kernel_spmd(nc, in_maps, list(range(N_CORES)))
    out = np.concatenate([res.results[i]["out"] for i in range(N_CORES)], axis=0)
    s_new = np.concatenate([res.results[i]["s_new"] for i in range(N_CORES)], axis=0)
    return np.concatenate([out.reshape(-1), s_new.reshape(-1)], axis=0)
